# revision 7
# baseline (speedup 1.0000x reference)
"""Trainium2 Bass kernel for nn_AttentionLayer (DIN-style attention MLP).

Per batch row b (B=2048, T=200, D=64, H1=80, H2=40):
  info = [q, k, q-k, q*k];  y1 = info @ W1 + b1
       = k @ (W1b-W1c) + (q*k) @ W1d + (q @ (W1a+W1c) + b1)
  h1 = prelu(y1, a1);  y2 = h1 @ W2 + b2;  h2 = prelu(y2, a2)
  logits = h2 @ Wf + bf;  w = softmax(logits);  out = w @ v

Pure data-parallel across 8 cores (256 batches each); all matmuls fp16
(end-to-end rel-err ~3e-4). Feature-major layout: k.T via SWDGE cast-DMA
(fp32->f16) + on-chip column duplication + HWDGE xbar DMA-transpose of
128x128 blocks (partitions 0-63 = k.T, 64-127 = copy scaled in-place by q
to form (q*k).T). mm1/mm2 are W-stationary f16 streams (N=400); PReLU is a
single fused custom DVE op (select(t>0,t,t*alpha), t = psum + per-batch
bias via PageIdx). Stage-2 matmul pairs pack PSUM partitions 0-39/64-103
(tile_position) so one PReLU covers two chunks. Logits use h2-stationary
matmuls (100-row slices, N=1) landing across PSUM partitions; softmax runs
in [batch, t] layout after a small PE transpose + reshape DMA. Final w @ v
uses v tiles as stationary (N=1), accumulating out.T columns in PSUM.
"""

import numpy as np

B, T, D = 2048, 200, 64
H1, H2 = 80, 40
N_CORES = 8
BC = B // N_CORES          # 256 batches per core
SC_B = 16                  # batches per super-chunk
N_SC = BC // SC_B          # 16 super-chunks
SC_ROWS = SC_B * T         # 3200 rows
N_BLK = SC_ROWS // 128     # 25 transpose blocks per super-chunk
CH_B = 2                   # batches per chunk
CH_ROWS = CH_B * T         # 400
N_CH = SC_B // CH_B        # 8 chunks per super-chunk
LG_SL = 100                # logits slice rows
N_LG = SC_ROWS // LG_SL    # 32 logit slices per super-chunk

_cache = {}


def _register_prelu_op():
    import concourse.dve_ops as dve_ops
    from concourse.dve_ops import DveOp, OPS
    from concourse.dve_spec import Spec, Src0, Src1, C0, C1, Zero, select, lower, PageIdx
    from concourse.dve_uop import DveOpSpec

    if "PRELU_PG_ANT" in dve_ops._SUB_OPCODE_FOR_NAME:
        for op in OPS:
            if op.name == "PRELU_PG_ANT":
                return op
    t = Src0 + PageIdx(C0, C1)
    spec = Spec(body=select(t > Zero, t, t * Src1))
    opcode = dve_ops._CUSTOM_DVE_ROW_BASE + len(OPS)
    shas = {}
    for ver in ("v3", "v4"):
        r = DveOpSpec(name="PRELU_PG_ANT", opcode=opcode,
                      uops=lower(spec, ver=ver), rd1_en=True)
        shas[ver] = r.sha(ver)
    op = DveOp("PRELU_PG_ANT", spec, subdim=True, uops_sha=shas)
    OPS.append(op)
    dve_ops._SUB_OPCODE_FOR_NAME["PRELU_PG_ANT"] = opcode
    return op


def _build():
    if "nc" in _cache:
        return _cache["nc"]
    import concourse.bacc as bacc
    import concourse.mybir as mybir
    import concourse.tile as tile
    from concourse import masks

    PRELU = _register_prelu_op()
    f16 = mybir.dt.float16
    f32 = mybir.dt.float32
    AF = mybir.ActivationFunctionType
    ALU = mybir.AluOpType
    AX = mybir.AxisListType

    nc = bacc.Bacc("TRN2", target_bir_lowering=False, debug=False,
                   num_devices=N_CORES)

    q_d = nc.dram_tensor("q", [BC, D], f32, kind="ExternalInput")
    k_d = nc.dram_tensor("k", [BC, T, D], f32, kind="ExternalInput")
    v_d = nc.dram_tensor("v", [BC, T, D], f32, kind="ExternalInput")
    W1_d = nc.dram_tensor("W1", [4 * D, H1], f32, kind="ExternalInput")
    b1_d = nc.dram_tensor("b1", [H1], f32, kind="ExternalInput")
    a1_d = nc.dram_tensor("a1", [T, H1], f32, kind="ExternalInput")
    W2_d = nc.dram_tensor("W2", [H1, H2], f32, kind="ExternalInput")
    b2_d = nc.dram_tensor("b2", [H2], f32, kind="ExternalInput")
    a2_d = nc.dram_tensor("a2", [T, H2], f32, kind="ExternalInput")
    Wf_d = nc.dram_tensor("Wf", [H2, 1], f32, kind="ExternalInput")
    bf_d = nc.dram_tensor("bf", [1], f32, kind="ExternalInput")
    out_d = nc.dram_tensor("out", [BC, D], f32, kind="ExternalOutput")

    kv = k_d.ap().rearrange("b t d -> (b t) d").rearrange("(n p) d -> p n d", p=128)
    vv = v_d.ap().rearrange("b t d -> (b t) d")

    with tile.TileContext(nc) as tc:
        with (
            tc.tile_pool(name="const", bufs=1) as cpool,
            tc.tile_pool(name="kb", bufs=2) as kbpool,
            tc.tile_pool(name="kt", bufs=2) as ktpool,
            tc.tile_pool(name="vb", bufs=2) as vbpool,
            tc.tile_pool(name="h", bufs=4) as hpool,
            tc.tile_pool(name="sm", bufs=2) as smpool,
            tc.tile_pool(name="p1", bufs=2, space="PSUM") as p1pool,
            tc.tile_pool(name="p2", bufs=2, space="PSUM") as p2pool,
            tc.tile_pool(name="pL", bufs=2, space="PSUM") as pLpool,
            tc.tile_pool(name="pv", bufs=1, space="PSUM") as pvpool,
            tc.tile_pool(name="ps", bufs=1, space="PSUM") as pspool,
        ):
            # ======== one-time setup ========
            ident32 = cpool.tile([128, 128], f32)
            masks.make_identity(nc, ident32[:])
            ident16 = cpool.tile([128, 128], f16)
            nc.vector.tensor_copy(ident16[:], ident32[:])

            # --- W1 pieces ---
            w1a = cpool.tile([64, H1], f32)
            w1b = cpool.tile([64, H1], f32)
            w1c = cpool.tile([64, H1], f32)
            nc.sync.dma_start(w1a[:], W1_d.ap()[0:64, :])
            nc.sync.dma_start(w1b[:], W1_d.ap()[64:128, :])
            nc.sync.dma_start(w1c[:], W1_d.ap()[128:192, :])
            w1s16 = cpool.tile([128, H1], f16)  # [W1b-W1c ; W1d]
            nc.vector.tensor_tensor(w1s16[0:64, :], w1b[:], w1c[:], ALU.subtract)
            nc.gpsimd.dma_start(w1s16[64:128, :], W1_d.ap()[192:256, :])
            w1ac = cpool.tile([64, H1], f32)
            nc.vector.tensor_tensor(w1ac[:], w1a[:], w1c[:], ALU.add)

            w2s16 = cpool.tile([H1, H2], f16)
            nc.gpsimd.dma_start(w2s16[:], W2_d.ap())
            wf16 = cpool.tile([104, 1], f16)
            nc.gpsimd.dma_start(wf16[0:H2, :], Wf_d.ap())
            nc.gpsimd.dma_start(wf16[64:64 + H2, :], Wf_d.ap())
            b2col = cpool.tile([128, 1], f32)
            nc.vector.memset(b2col[:], 0.0)
            nc.sync.dma_start(b2col[0:H2, 0], b2_d.ap())
            nc.sync.dma_start(b2col[64:64 + H2, 0], b2_d.ap())
            b1col = cpool.tile([H1, 1], f32)
            nc.sync.dma_start(b1col[:, 0], b1_d.ap())
            zcol = cpool.tile([128, 1], f32)
            nc.vector.memset(zcol[:], 0.0)
            n8col = cpool.tile([LG_SL, 1], f32)
            nc.vector.memset(n8col[:], -8.0)

            # --- qT (f32), duplicated onto partitions 64-127 ---
            qTall = cpool.tile([128, BC], f32)
            for g in range(2):
                qn = cpool.tile([128, D], f32, tag=f"qn{g}", name=f"qn{g}")
                nc.sync.dma_start(qn[:], q_d.ap()[g * 128:(g + 1) * 128, :])
                pt = pspool.tile([128, 512], f32, tag="setup", name=f"ptq{g}")
                nc.tensor.transpose(pt[0:D, 0:128], qn[:], ident32[:])
                nc.vector.tensor_copy(qTall[0:D, g * 128:(g + 1) * 128],
                                      pt[0:D, 0:128])
            nc.sync.dma_start(qTall[64:64 + D, :], qTall[0:D, :])

            # --- C1T [H1, BC] = (q @ W1ac).T + b1; delta cols for PageIdx ---
            C1T = cpool.tile([H1, BC], f32)
            for g in range(2):
                pt = pspool.tile([128, 512], f32, tag="setup", name=f"ptc{g}")
                nc.tensor.matmul(pt[0:H1, 0:128], w1ac[:],
                                 qTall[0:D, g * 128:(g + 1) * 128],
                                 start=True, stop=True)
                nc.vector.tensor_scalar(C1T[:, g * 128:(g + 1) * 128],
                                        pt[0:H1, 0:128], b1col[:], None, ALU.add)
            dC1T = cpool.tile([H1, BC // 2], f32)
            c1v = C1T[:].rearrange("p (c two) -> p c two", two=2)
            nc.vector.tensor_tensor(dC1T[:], c1v[:, :, 1], c1v[:, :, 0],
                                    ALU.subtract)

            # --- alpha tiles: a1T rep x2 [H1, 400]; a2T pair [104, 400] ---
            a1rep = cpool.tile([H1, CH_ROWS], f16)
            a2rep = cpool.tile([104, CH_ROWS], f16)
            for (ad, hh, dst) in ((a1_d, H1, a1rep), (a2_d, H2, a2rep)):
                an = cpool.tile([128, hh], f32, tag=f"an{hh}", name=f"an{hh}")
                an2 = cpool.tile([72, hh], f32, tag=f"an2{hh}", name=f"an2{hh}")
                nc.sync.dma_start(an[:], ad.ap()[0:128, :])
                nc.sync.dma_start(an2[:], ad.ap()[128:200, :])
                pt = pspool.tile([128, 512], f32, tag="setup", name=f"pta{hh}")
                nc.tensor.transpose(pt[0:hh, 0:128], an[:], ident32[:])
                nc.tensor.transpose(pt[0:hh, 128:200], an2[:], ident32[0:72, 0:72])
                nc.vector.tensor_copy(dst[0:hh, 0:T], pt[0:hh, 0:T])
                nc.vector.tensor_copy(dst[0:hh, T:2 * T], pt[0:hh, 0:T])
                if dst is a2rep:
                    nc.sync.dma_start(dst[64:64 + hh, :], dst[0:hh, :])

            pv = pvpool.tile([128, BC], f32)  # persistent out.T accumulator

            # ======== main loop over super-chunks ========
            for s in range(N_SC):
                # ---- k: cast-load, duplicate cols, transpose blocks ----
                knat = kbpool.tile([128, N_BLK * 128], f16, tag="knat")
                knv = knat[:].rearrange("p (n c) -> p n c", c=128)
                for blk in range(N_BLK):
                    nc.gpsimd.dma_start(knv[:, blk, 0:64],
                                        kv[:, s * N_BLK + blk, :])
                nc.sync.dma_start(knv[:, :, 64:128], knv[:, :, 0:64])
                ktw = ktpool.tile([128, SC_ROWS], f16, tag="ktw")
                for blk in range(N_BLK):
                    nc.sync.dma_start(ktw[:, blk * 128:(blk + 1) * 128],
                                      knv[:, blk, :], transpose=True)
                # qk in place on partitions 64-127
                for b in range(SC_B):
                    bg = s * SC_B + b
                    nc.vector.tensor_scalar(
                        ktw[64:128, b * T:(b + 1) * T],
                        ktw[64:128, b * T:(b + 1) * T],
                        qTall[64:128, bg:bg + 1], None, ALU.mult)

                # ---- v: cast-load into padded blocks (128 cols / batch-half) --
                vblk = vbpool.tile([128, SC_B * 256], f16, tag="vblk")
                for b in range(SC_B):
                    bg = s * SC_B + b
                    nc.gpsimd.dma_start(vblk[:, b * 256: b * 256 + 64],
                                        vv[bg * T: bg * T + 128, :])
                    nc.gpsimd.dma_start(vblk[0:72, b * 256 + 128: b * 256 + 192],
                                        vv[bg * T + 128: bg * T + 200, :])

                hhsc = []
                for c in range(0, N_CH, 2):
                    h1pair = []
                    for cc in (c, c + 1):
                        p1 = p1pool.tile([H1, CH_ROWS], f32, tag="p1",
                                         name=f"p1_{s}_{cc}")
                        nc.tensor.matmul(p1[:], w1s16[:],
                                         ktw[:, cc * CH_ROWS:(cc + 1) * CH_ROWS],
                                         start=True, stop=True)
                        h1 = hpool.tile([H1, CH_ROWS], f16, tag="h1",
                                        name=f"h1_{s}_{cc}")
                        bg = s * SC_B + cc * CH_B
                        nc.vector._custom_dve(
                            PRELU,
                            out=h1[:].rearrange("p (s n) -> p s n", s=2),
                            in0=p1[:].rearrange("p (s n) -> p s n", s=2),
                            in1=a1rep[:],
                            s0=C1T[:, bg:bg + 1],
                            s1=dC1T[:, bg // 2: bg // 2 + 1])
                        h1pair.append(h1)
                    p2 = p2pool.tile([104, CH_ROWS], f32, tag="p2",
                                     name=f"p2_{s}_{c}")
                    nc.tensor.matmul(p2[0:H2, :], w2s16[:], h1pair[0][:],
                                     start=True, stop=True)
                    nc.tensor.matmul(p2[64:64 + H2, :], w2s16[:], h1pair[1][:],
                                     start=True, stop=True, tile_position=(0, 64))
                    h2 = hpool.tile([104, CH_ROWS], f16, tag="h2",
                                    name=f"h2_{s}_{c}")
                    nc.vector._custom_dve(
                        PRELU,
                        out=h2[:].rearrange("p (s n) -> p s n", s=2),
                        in0=p2[:].rearrange("p (s n) -> p s n", s=2),
                        in1=a2rep[:],
                        s0=b2col[0:104], s1=zcol[0:104])
                    hhsc.append(h2)

                # ---- logits: h2-stationary matmuls, 100-row slices ----
                pL = pLpool.tile([LG_SL, N_LG], f32, tag="pL", name=f"pL{s}")
                for c2 in range(N_CH // 2):
                    h2 = hhsc[c2]
                    for half, base in ((0, 0), (1, 64)):
                        for sl in range(4):
                            gsl = c2 * 8 + half * 4 + sl
                            nc.tensor.matmul(
                                pL[:, gsl:gsl + 1],
                                h2[base:base + H2, sl * LG_SL:(sl + 1) * LG_SL],
                                wf16[base:base + H2, :], start=True, stop=True)

                # ---- softmax in [b, t] layout ----
                u16 = smpool.tile([LG_SL, N_LG], f16, tag="u16", name=f"u{s}")
                nc.scalar.activation(u16[:], pL[:], AF.Exp, bias=n8col[:])
                psm = pspool.tile([128, 1024], f16, tag="setup", name=f"psm{s}")
                nc.tensor.transpose(psm[0:N_LG, 0:LG_SL], u16[:], ident16[0:LG_SL, 0:LG_SL])
                uT = smpool.tile([N_LG, LG_SL], f16, tag="uT", name=f"uT{s}")
                nc.vector.tensor_copy(uT[:], psm[0:N_LG, 0:LG_SL])
                unat = smpool.tile([SC_B, T], f16, tag="unat", name=f"un{s}")
                unat_v = unat[:].rearrange("b (two t) -> b two t", two=2)
                uT_v = uT[:].rearrange("(b two) t -> b two t", b=SC_B)
                nc.sync.dma_start(unat_v[:, 0, :], uT_v[:, 0, :])
                nc.sync.dma_start(unat_v[:, 1, :], uT_v[:, 1, :])
                ssum = smpool.tile([SC_B, 1], f32, tag="ssum", name=f"ss{s}")
                nc.vector.tensor_reduce(ssum[:], unat[:], AX.X, ALU.add)
                rs = smpool.tile([SC_B, 1], f32, tag="rs", name=f"rs{s}")
                nc.vector.reciprocal(rs[:], ssum[:])
                wts = smpool.tile([SC_B, T], f16, tag="wts", name=f"w{s}")
                nc.vector.tensor_scalar(wts[:], unat[:], rs[:], None, ALU.mult)
                psw = pspool.tile([128, 1024], f16, tag="setup", name=f"psw{s}")
                nc.tensor.transpose(psw[0:128, 0:SC_B], wts[:, 0:128], ident16[0:SC_B, 0:SC_B])
                nc.tensor.transpose(psw[0:72, 128:128 + SC_B], wts[:, 128:200],
                                    ident16[0:SC_B, 0:SC_B])
                wT = smpool.tile([128, 2 * SC_B], f16, tag="wT", name=f"wT{s}")
                nc.vector.tensor_copy(wT[:, 0:SC_B], psw[:, 0:SC_B])
                nc.vector.tensor_copy(wT[0:72, SC_B:2 * SC_B],
                                      psw[0:72, 128:128 + SC_B])

                # ---- out accumulation: v-stationary matmuls ----
                for b in range(SC_B):
                    bg = s * SC_B + b
                    nc.tensor.matmul(pv[:, bg:bg + 1],
                                     vblk[:, b * 256:b * 256 + 128],
                                     wT[:, b:b + 1], start=True, stop=False)
                    nc.tensor.matmul(pv[:, bg:bg + 1],
                                     vblk[0:72, b * 256 + 128:b * 256 + 256],
                                     wT[0:72, SC_B + b:SC_B + b + 1],
                                     start=False, stop=True)

            # ======== final: out.T [64, BC] -> [BC, 64] ========
            oT = cpool.tile([64, BC], f32)
            nc.scalar.copy(oT[:], pv[0:64, :])
            for g in range(2):
                pt = pspool.tile([128, 512], f32, tag="setup", name=f"pto{g}")
                nc.tensor.transpose(pt[0:128, 0:64],
                                    oT[:, g * 128:(g + 1) * 128], ident32[0:64, 0:64])
                onat = cpool.tile([128, 64], f32, tag=f"onat{g}", name=f"onat{g}")
                nc.vector.tensor_copy(onat[:], pt[0:128, 0:64])
                nc.sync.dma_start(out_d.ap()[g * 128:(g + 1) * 128, :], onat[:])

    nc.compile()
    _cache["nc"] = nc
    return nc


def kernel(q, k, v, W1, b1, a1, W2, b2, a2, Wf, bf):
    from concourse.bass_utils import run_bass_kernel_spmd

    nc = _build()
    q = np.ascontiguousarray(np.asarray(q, dtype=np.float32))
    k = np.ascontiguousarray(np.asarray(k, dtype=np.float32))
    v = np.ascontiguousarray(np.asarray(v, dtype=np.float32))
    shared = {n: np.ascontiguousarray(np.asarray(x, np.float32))
              for n, x in (("W1", W1), ("b1", b1), ("a1", a1), ("W2", W2),
                           ("b2", b2), ("a2", a2), ("Wf", Wf), ("bf", bf))}
    in_maps = []
    for c in range(N_CORES):
        sl = slice(c * BC, (c + 1) * BC)
        m = {"q": q[sl], "k": k[sl], "v": v[sl]}
        m.update(shared)
        in_maps.append(m)
    res = run_bass_kernel_spmd(nc, in_maps, core_ids=list(range(N_CORES)))
    out = np.empty((B, D), dtype=np.float32)
    for c in range(N_CORES):
        out[c * BC:(c + 1) * BC] = res.results[c]["out"]
    return out


# revision 8
# speedup vs baseline: 1.7145x; 1.7145x over previous
"""Trainium2 Bass kernel for nn_AttentionLayer (DIN-style attention MLP).

Per batch row b (B=2048, T=200, D=64, H1=80, H2=40):
  info = [q, k, q-k, q*k];  y1 = info @ W1 + b1
       = k @ (W1b-W1c) + (q*k) @ W1d + (q @ (W1a+W1c) + b1)
  h1 = prelu(y1, a1);  y2 = h1 @ W2 + b2;  h2 = prelu(y2, a2)
  logits = h2 @ Wf + bf;  w = softmax(logits);  out = w @ v

Pure data-parallel across 8 cores (256 batches each); all matmuls fp16
(end-to-end rel-err ~3e-4). Feature-major layout: k.T via SWDGE cast-DMA
(fp32->f16) + on-chip column duplication + HWDGE xbar DMA-transpose of
128x128 blocks (partitions 0-63 = k.T, 64-127 = copy scaled in-place by q
to form (q*k).T). mm1/mm2 are W-stationary f16 streams (N=400); PReLU is a
single fused custom DVE op (select(t>0,t,t*alpha), t = psum + per-batch
bias via PageIdx). Stage-2 matmul pairs pack PSUM partitions 0-39/64-103
(tile_position) so one PReLU covers two chunks. Logits use h2-stationary
matmuls (100-row slices, N=1) landing across PSUM partitions; softmax runs
in [batch, t] layout after a small PE transpose + reshape DMA. Final w @ v
uses v tiles as stationary (N=1), accumulating out.T columns in PSUM.
"""

import numpy as np

B, T, D = 2048, 200, 64
H1, H2 = 80, 40
N_CORES = 8
BC = B // N_CORES          # 256 batches per core
SC_B = 16                  # batches per super-chunk
N_SC = BC // SC_B          # 16 super-chunks
SC_ROWS = SC_B * T         # 3200 rows
N_BLK = SC_ROWS // 128     # 25 transpose blocks per super-chunk
CH_B = 2                   # batches per chunk
CH_ROWS = CH_B * T         # 400
N_CH = SC_B // CH_B        # 8 chunks per super-chunk
LG_SL = 100                # logits slice rows
N_LG = SC_ROWS // LG_SL    # 32 logit slices per super-chunk

_cache = {}


def _register_prelu_op():
    import concourse.dve_ops as dve_ops
    from concourse.dve_ops import DveOp, OPS
    from concourse.dve_spec import Spec, Src0, Src1, C0, C1, Zero, select, lower, PageIdx
    from concourse.dve_uop import DveOpSpec

    if "PRELU_PG_ANT" in dve_ops._SUB_OPCODE_FOR_NAME:
        for op in OPS:
            if op.name == "PRELU_PG_ANT":
                return op
    t = Src0 + PageIdx(C0, C1)
    spec = Spec(body=select(t > Zero, t, t * Src1))
    opcode = dve_ops._CUSTOM_DVE_ROW_BASE + len(OPS)
    shas = {}
    for ver in ("v3", "v4"):
        r = DveOpSpec(name="PRELU_PG_ANT", opcode=opcode,
                      uops=lower(spec, ver=ver), rd1_en=True)
        shas[ver] = r.sha(ver)
    op = DveOp("PRELU_PG_ANT", spec, subdim=True, uops_sha=shas)
    OPS.append(op)
    dve_ops._SUB_OPCODE_FOR_NAME["PRELU_PG_ANT"] = opcode
    return op


def _build():
    if "nc" in _cache:
        return _cache["nc"]
    import concourse.bacc as bacc
    import concourse.mybir as mybir
    import concourse.tile as tile
    from concourse import masks

    PRELU = _register_prelu_op()
    f16 = mybir.dt.float16
    f32 = mybir.dt.float32
    AF = mybir.ActivationFunctionType
    ALU = mybir.AluOpType
    AX = mybir.AxisListType

    nc = bacc.Bacc("TRN2", target_bir_lowering=False, debug=False,
                   num_devices=N_CORES)

    q_d = nc.dram_tensor("q", [BC, D], f32, kind="ExternalInput")
    k_d = nc.dram_tensor("k", [BC, T, D], f32, kind="ExternalInput")
    v_d = nc.dram_tensor("v", [BC, T, D], f32, kind="ExternalInput")
    W1_d = nc.dram_tensor("W1", [4 * D, H1], f32, kind="ExternalInput")
    b1_d = nc.dram_tensor("b1", [H1], f32, kind="ExternalInput")
    a1_d = nc.dram_tensor("a1", [T, H1], f32, kind="ExternalInput")
    W2_d = nc.dram_tensor("W2", [H1, H2], f32, kind="ExternalInput")
    b2_d = nc.dram_tensor("b2", [H2], f32, kind="ExternalInput")
    a2_d = nc.dram_tensor("a2", [T, H2], f32, kind="ExternalInput")
    Wf_d = nc.dram_tensor("Wf", [H2, 1], f32, kind="ExternalInput")
    bf_d = nc.dram_tensor("bf", [1], f32, kind="ExternalInput")
    out_d = nc.dram_tensor("out", [BC, D], f32, kind="ExternalOutput")

    kv = k_d.ap().rearrange("b t d -> (b t) d").rearrange("(n p) d -> p n d", p=128)
    vv = v_d.ap().rearrange("b t d -> (b t) d")

    with tile.TileContext(nc) as tc:
        with (
            tc.tile_pool(name="const", bufs=1) as cpool,
            tc.tile_pool(name="kb", bufs=2) as kbpool,
            tc.tile_pool(name="kt", bufs=2) as ktpool,
            tc.tile_pool(name="vb", bufs=2) as vbpool,
            tc.tile_pool(name="h", bufs=4) as hpool,
            tc.tile_pool(name="sm", bufs=2) as smpool,
            tc.tile_pool(name="p1", bufs=2, space="PSUM") as p1pool,
            tc.tile_pool(name="p2", bufs=2, space="PSUM") as p2pool,
            tc.tile_pool(name="pL", bufs=2, space="PSUM") as pLpool,
            tc.tile_pool(name="pv", bufs=1, space="PSUM") as pvpool,
            tc.tile_pool(name="ps", bufs=1, space="PSUM") as pspool,
        ):
            # ======== one-time setup ========
            ident32 = cpool.tile([128, 128], f32)
            masks.make_identity(nc, ident32[:])
            ident16 = cpool.tile([128, 128], f16)
            nc.vector.tensor_copy(ident16[:], ident32[:])

            # --- W1 pieces ---
            w1a = cpool.tile([64, H1], f32)
            w1b = cpool.tile([64, H1], f32)
            w1c = cpool.tile([64, H1], f32)
            nc.sync.dma_start(w1a[:], W1_d.ap()[0:64, :])
            nc.sync.dma_start(w1b[:], W1_d.ap()[64:128, :])
            nc.sync.dma_start(w1c[:], W1_d.ap()[128:192, :])
            w1s16 = cpool.tile([128, H1], f16)  # [W1b-W1c ; W1d]
            nc.vector.tensor_tensor(w1s16[0:64, :], w1b[:], w1c[:], ALU.subtract)
            nc.gpsimd.dma_start(w1s16[64:128, :], W1_d.ap()[192:256, :])
            w1ac = cpool.tile([64, H1], f32)
            nc.vector.tensor_tensor(w1ac[:], w1a[:], w1c[:], ALU.add)

            w2s16 = cpool.tile([H1, H2], f16)
            nc.gpsimd.dma_start(w2s16[:], W2_d.ap())
            wf16 = cpool.tile([104, 1], f16)
            nc.gpsimd.dma_start(wf16[0:H2, :], Wf_d.ap())
            nc.gpsimd.dma_start(wf16[64:64 + H2, :], Wf_d.ap())
            b2col = cpool.tile([128, 1], f32)
            nc.vector.memset(b2col[:], 0.0)
            nc.sync.dma_start(b2col[0:H2, 0], b2_d.ap())
            nc.sync.dma_start(b2col[64:64 + H2, 0], b2_d.ap())
            b1col = cpool.tile([H1, 1], f32)
            nc.sync.dma_start(b1col[:, 0], b1_d.ap())
            zcol = cpool.tile([128, 1], f32)
            nc.vector.memset(zcol[:], 0.0)
            n8col = cpool.tile([LG_SL, 1], f32)
            nc.vector.memset(n8col[:], -8.0)

            # --- qT (f32), duplicated onto partitions 64-127 ---
            qTall = cpool.tile([128, BC], f32)
            for g in range(2):
                qn = cpool.tile([128, D], f32, tag=f"qn{g}", name=f"qn{g}")
                nc.sync.dma_start(qn[:], q_d.ap()[g * 128:(g + 1) * 128, :])
                pt = pspool.tile([128, 512], f32, tag="setup", name=f"ptq{g}")
                nc.tensor.transpose(pt[0:D, 0:128], qn[:], ident32[:])
                nc.vector.tensor_copy(qTall[0:D, g * 128:(g + 1) * 128],
                                      pt[0:D, 0:128])
            nc.sync.dma_start(qTall[64:64 + D, :], qTall[0:D, :])

            # --- C1T [H1, BC] = (q @ W1ac).T + b1; delta cols for PageIdx ---
            C1T = cpool.tile([H1, BC], f32)
            for g in range(2):
                pt = pspool.tile([128, 512], f32, tag="setup", name=f"ptc{g}")
                nc.tensor.matmul(pt[0:H1, 0:128], w1ac[:],
                                 qTall[0:D, g * 128:(g + 1) * 128],
                                 start=True, stop=True)
                nc.vector.tensor_scalar(C1T[:, g * 128:(g + 1) * 128],
                                        pt[0:H1, 0:128], b1col[:], None, ALU.add)
            dC1T = cpool.tile([H1, BC // 2], f32)
            c1v = C1T[:].rearrange("p (c two) -> p c two", two=2)
            nc.vector.tensor_tensor(dC1T[:], c1v[:, :, 1], c1v[:, :, 0],
                                    ALU.subtract)

            # --- alpha tiles: a1T rep x2 [H1, 400]; a2T pair [104, 400] ---
            a1rep = cpool.tile([H1, CH_ROWS], f16)
            a2rep = cpool.tile([104, CH_ROWS], f16)
            for (ad, hh, dst) in ((a1_d, H1, a1rep), (a2_d, H2, a2rep)):
                an = cpool.tile([128, hh], f32, tag=f"an{hh}", name=f"an{hh}")
                an2 = cpool.tile([72, hh], f32, tag=f"an2{hh}", name=f"an2{hh}")
                nc.sync.dma_start(an[:], ad.ap()[0:128, :])
                nc.sync.dma_start(an2[:], ad.ap()[128:200, :])
                pt = pspool.tile([128, 512], f32, tag="setup", name=f"pta{hh}")
                nc.tensor.transpose(pt[0:hh, 0:128], an[:], ident32[:])
                nc.tensor.transpose(pt[0:hh, 128:200], an2[:], ident32[0:72, 0:72])
                nc.vector.tensor_copy(dst[0:hh, 0:T], pt[0:hh, 0:T])
                nc.vector.tensor_copy(dst[0:hh, T:2 * T], pt[0:hh, 0:T])
                if dst is a2rep:
                    nc.sync.dma_start(dst[64:64 + hh, :], dst[0:hh, :])

            pv = pvpool.tile([128, BC], f32)  # persistent out.T accumulator

            # ======== main loop over super-chunks ========
            for s in range(N_SC):
                # ---- k: cast-load, duplicate cols, transpose blocks ----
                knat = kbpool.tile([128, N_BLK * 128], f16, tag="knat")
                knv = knat[:].rearrange("p (n c) -> p n c", c=128)
                nc.gpsimd.dma_start(knv[:, :, 0:64],
                                    kv[:, s * N_BLK:(s + 1) * N_BLK, :])
                ktw = ktpool.tile([128, SC_ROWS], f16, tag="ktw")
                for blk in range(N_BLK):
                    nc.sync.dma_start(ktw[:, blk * 128:(blk + 1) * 128],
                                      knv[:, blk, :], transpose=True)
                nc.sync.dma_start(ktw[64:128, :], ktw[0:64, :])
                # qk in place on partitions 64-127
                for b in range(SC_B):
                    bg = s * SC_B + b
                    nc.vector.tensor_scalar(
                        ktw[64:128, b * T:(b + 1) * T],
                        ktw[64:128, b * T:(b + 1) * T],
                        qTall[64:128, bg:bg + 1], None, ALU.mult)

                # ---- v: cast-load into padded blocks (128 cols / batch-half) --
                vblk = vbpool.tile([128, SC_B * 256], f16, tag="vblk")
                vbv = vblk[:].rearrange("p (b c) -> p b c", c=256)
                vsrc = v_d.ap()[s * SC_B:(s + 1) * SC_B, :, :]
                nc.gpsimd.dma_start(
                    vbv[:, :, 0:64],
                    vsrc[:, 0:128, :].rearrange("b p d -> p b d"))
                nc.gpsimd.dma_start(
                    vbv[0:72, :, 128:192],
                    vsrc[:, 128:200, :].rearrange("b p d -> p b d"))

                hhsc = []
                for c in range(0, N_CH, 2):
                    h1pair = []
                    for cc in (c, c + 1):
                        p1 = p1pool.tile([H1, CH_ROWS], f32, tag="p1",
                                         name=f"p1_{s}_{cc}")
                        nc.tensor.matmul(p1[:], w1s16[:],
                                         ktw[:, cc * CH_ROWS:(cc + 1) * CH_ROWS],
                                         start=True, stop=True)
                        h1 = hpool.tile([H1, CH_ROWS], f16, tag="h1",
                                        name=f"h1_{s}_{cc}")
                        bg = s * SC_B + cc * CH_B
                        nc.vector._custom_dve(
                            PRELU,
                            out=h1[:].rearrange("p (s n) -> p s n", s=2),
                            in0=p1[:].rearrange("p (s n) -> p s n", s=2),
                            in1=a1rep[:],
                            s0=C1T[:, bg:bg + 1],
                            s1=dC1T[:, bg // 2: bg // 2 + 1])
                        h1pair.append(h1)
                    p2 = p2pool.tile([104, CH_ROWS], f32, tag="p2",
                                     name=f"p2_{s}_{c}")
                    nc.tensor.matmul(p2[0:H2, :], w2s16[:], h1pair[0][:],
                                     start=True, stop=True)
                    nc.tensor.matmul(p2[64:64 + H2, :], w2s16[:], h1pair[1][:],
                                     start=True, stop=True, tile_position=(0, 64))
                    h2 = hpool.tile([104, CH_ROWS], f16, tag="h2",
                                    name=f"h2_{s}_{c}")
                    nc.vector._custom_dve(
                        PRELU,
                        out=h2[:].rearrange("p (s n) -> p s n", s=2),
                        in0=p2[:].rearrange("p (s n) -> p s n", s=2),
                        in1=a2rep[:],
                        s0=b2col[0:104], s1=zcol[0:104])
                    hhsc.append(h2)

                # ---- logits: h2-stationary matmuls, 100-row slices ----
                pL = pLpool.tile([LG_SL, N_LG], f32, tag="pL", name=f"pL{s}")
                for c2 in range(N_CH // 2):
                    h2 = hhsc[c2]
                    for half, base in ((0, 0), (1, 64)):
                        for sl in range(4):
                            gsl = c2 * 8 + half * 4 + sl
                            nc.tensor.matmul(
                                pL[:, gsl:gsl + 1],
                                h2[base:base + H2, sl * LG_SL:(sl + 1) * LG_SL],
                                wf16[base:base + H2, :], start=True, stop=True)

                # ---- softmax in [b, t] layout ----
                u16 = smpool.tile([LG_SL, N_LG], f16, tag="u16", name=f"u{s}")
                nc.scalar.activation(u16[:], pL[:], AF.Exp, bias=n8col[:])
                psm = pspool.tile([128, 1024], f16, tag="setup", name=f"psm{s}")
                nc.tensor.transpose(psm[0:N_LG, 0:LG_SL], u16[:], ident16[0:LG_SL, 0:LG_SL])
                uT = smpool.tile([N_LG, LG_SL], f16, tag="uT", name=f"uT{s}")
                nc.vector.tensor_copy(uT[:], psm[0:N_LG, 0:LG_SL])
                unat = smpool.tile([SC_B, T], f16, tag="unat", name=f"un{s}")
                unat_v = unat[:].rearrange("b (two t) -> b two t", two=2)
                uT_v = uT[:].rearrange("(b two) t -> b two t", b=SC_B)
                nc.sync.dma_start(unat_v[:, 0, :], uT_v[:, 0, :])
                nc.sync.dma_start(unat_v[:, 1, :], uT_v[:, 1, :])
                ssum = smpool.tile([SC_B, 1], f32, tag="ssum", name=f"ss{s}")
                nc.vector.tensor_reduce(ssum[:], unat[:], AX.X, ALU.add)
                rs = smpool.tile([SC_B, 1], f32, tag="rs", name=f"rs{s}")
                nc.vector.reciprocal(rs[:], ssum[:])
                wts = smpool.tile([SC_B, T], f16, tag="wts", name=f"w{s}")
                nc.vector.tensor_scalar(wts[:], unat[:], rs[:], None, ALU.mult)
                psw = pspool.tile([128, 1024], f16, tag="setup", name=f"psw{s}")
                nc.tensor.transpose(psw[0:128, 0:SC_B], wts[:, 0:128], ident16[0:SC_B, 0:SC_B])
                nc.tensor.transpose(psw[0:72, 128:128 + SC_B], wts[:, 128:200],
                                    ident16[0:SC_B, 0:SC_B])
                wT = smpool.tile([128, 2 * SC_B], f16, tag="wT", name=f"wT{s}")
                nc.vector.tensor_copy(wT[:, 0:SC_B], psw[:, 0:SC_B])
                nc.vector.tensor_copy(wT[0:72, SC_B:2 * SC_B],
                                      psw[0:72, 128:128 + SC_B])

                # ---- out accumulation: v-stationary matmuls ----
                for b in range(SC_B):
                    bg = s * SC_B + b
                    nc.tensor.matmul(pv[:, bg:bg + 1],
                                     vblk[:, b * 256:b * 256 + 128],
                                     wT[:, b:b + 1], start=True, stop=False)
                    nc.tensor.matmul(pv[:, bg:bg + 1],
                                     vblk[0:72, b * 256 + 128:b * 256 + 256],
                                     wT[0:72, SC_B + b:SC_B + b + 1],
                                     start=False, stop=True)

            # ======== final: out.T [64, BC] -> [BC, 64] ========
            oT = cpool.tile([64, BC], f32)
            nc.scalar.copy(oT[:], pv[0:64, :])
            for g in range(2):
                pt = pspool.tile([128, 512], f32, tag="setup", name=f"pto{g}")
                nc.tensor.transpose(pt[0:128, 0:64],
                                    oT[:, g * 128:(g + 1) * 128], ident32[0:64, 0:64])
                onat = cpool.tile([128, 64], f32, tag=f"onat{g}", name=f"onat{g}")
                nc.vector.tensor_copy(onat[:], pt[0:128, 0:64])
                nc.sync.dma_start(out_d.ap()[g * 128:(g + 1) * 128, :], onat[:])

    nc.compile()
    _cache["nc"] = nc
    return nc


def kernel(q, k, v, W1, b1, a1, W2, b2, a2, Wf, bf):
    from concourse.bass_utils import run_bass_kernel_spmd

    nc = _build()
    q = np.ascontiguousarray(np.asarray(q, dtype=np.float32))
    k = np.ascontiguousarray(np.asarray(k, dtype=np.float32))
    v = np.ascontiguousarray(np.asarray(v, dtype=np.float32))
    shared = {n: np.ascontiguousarray(np.asarray(x, np.float32))
              for n, x in (("W1", W1), ("b1", b1), ("a1", a1), ("W2", W2),
                           ("b2", b2), ("a2", a2), ("Wf", Wf), ("bf", bf))}
    in_maps = []
    for c in range(N_CORES):
        sl = slice(c * BC, (c + 1) * BC)
        m = {"q": q[sl], "k": k[sl], "v": v[sl]}
        m.update(shared)
        in_maps.append(m)
    res = run_bass_kernel_spmd(nc, in_maps, core_ids=list(range(N_CORES)))
    out = np.empty((B, D), dtype=np.float32)
    for c in range(N_CORES):
        out[c * BC:(c + 1) * BC] = res.results[c]["out"]
    return out


# revision 9
# speedup vs baseline: 4.4443x; 2.5922x over previous
"""Trainium2 Bass kernel for nn_AttentionLayer (DIN-style attention MLP).

Per batch row b (B=2048, T=200, D=64, H1=80, H2=40):
  info = [q, k, q-k, q*k];  y1 = info @ W1 + b1
       = k @ (W1b-W1c) + (q*k) @ W1d + (q @ (W1a+W1c) + b1)
  h1 = prelu(y1, a1);  y2 = h1 @ W2 + b2;  h2 = prelu(y2, a2)
  logits = h2 @ Wf + bf;  w = softmax(logits);  out = w @ v

Pure data-parallel across 8 cores (256 batches each); all matmuls fp16
(end-to-end rel-err ~3e-4). Feature-major layout: k.T via SWDGE cast-DMA
(fp32->f16) + on-chip column duplication + HWDGE xbar DMA-transpose of
128x128 blocks (partitions 0-63 = k.T, 64-127 = copy scaled in-place by q
to form (q*k).T). mm1/mm2 are W-stationary f16 streams (N=400); PReLU is a
single fused custom DVE op (select(t>0,t,t*alpha), t = psum + per-batch
bias via PageIdx). Stage-2 matmul pairs pack PSUM partitions 0-39/64-103
(tile_position) so one PReLU covers two chunks. Logits use h2-stationary
matmuls (100-row slices, N=1) landing across PSUM partitions; softmax runs
in [batch, t] layout after a small PE transpose + reshape DMA. Final w @ v
uses v tiles as stationary (N=1), accumulating out.T columns in PSUM.
"""

import numpy as np

B, T, D = 2048, 200, 64
H1, H2 = 80, 40
N_CORES = 8
BC = B // N_CORES          # 256 batches per core
SC_B = 16                  # batches per super-chunk
N_SC = BC // SC_B          # 16 super-chunks
SC_ROWS = SC_B * T         # 3200 rows
N_BLK = SC_ROWS // 128     # 25 transpose blocks per super-chunk
CH_B = 2                   # batches per chunk
CH_ROWS = CH_B * T         # 400
N_CH = SC_B // CH_B        # 8 chunks per super-chunk
LG_SL = 100                # logits slice rows
N_LG = SC_ROWS // LG_SL    # 32 logit slices per super-chunk

_cache = {}


def _register_prelu_op():
    import concourse.dve_ops as dve_ops
    from concourse.dve_ops import DveOp, OPS
    from concourse.dve_spec import Spec, Src0, Src1, C0, C1, Zero, select, lower, PageIdx
    from concourse.dve_uop import DveOpSpec

    if "PRELU_PG_ANT" in dve_ops._SUB_OPCODE_FOR_NAME:
        for op in OPS:
            if op.name == "PRELU_PG_ANT":
                return op
    t = Src0 + PageIdx(C0, C1)
    spec = Spec(body=select(t > Zero, t, t * Src1))
    opcode = dve_ops._CUSTOM_DVE_ROW_BASE + len(OPS)
    shas = {}
    for ver in ("v3", "v4"):
        r = DveOpSpec(name="PRELU_PG_ANT", opcode=opcode,
                      uops=lower(spec, ver=ver), rd1_en=True)
        shas[ver] = r.sha(ver)
    op = DveOp("PRELU_PG_ANT", spec, subdim=True, uops_sha=shas)
    OPS.append(op)
    dve_ops._SUB_OPCODE_FOR_NAME["PRELU_PG_ANT"] = opcode
    return op


def _build():
    if "nc" in _cache:
        return _cache["nc"]
    import concourse.bacc as bacc
    import concourse.mybir as mybir
    import concourse.tile as tile
    from concourse import masks

    PRELU = _register_prelu_op()
    f16 = mybir.dt.float16
    f32 = mybir.dt.float32
    AF = mybir.ActivationFunctionType
    ALU = mybir.AluOpType
    AX = mybir.AxisListType

    nc = bacc.Bacc("TRN2", target_bir_lowering=False, debug=False,
                   num_devices=N_CORES)

    q_d = nc.dram_tensor("q", [BC, D], f32, kind="ExternalInput")
    k_d = nc.dram_tensor("k", [BC, T, D], f32, kind="ExternalInput")
    v_d = nc.dram_tensor("v", [BC, T, D], f32, kind="ExternalInput")
    W1_d = nc.dram_tensor("W1", [4 * D, H1], f32, kind="ExternalInput")
    b1_d = nc.dram_tensor("b1", [H1], f32, kind="ExternalInput")
    a1_d = nc.dram_tensor("a1", [T, H1], f32, kind="ExternalInput")
    W2_d = nc.dram_tensor("W2", [H1, H2], f32, kind="ExternalInput")
    b2_d = nc.dram_tensor("b2", [H2], f32, kind="ExternalInput")
    a2_d = nc.dram_tensor("a2", [T, H2], f32, kind="ExternalInput")
    Wf_d = nc.dram_tensor("Wf", [H2, 1], f32, kind="ExternalInput")
    bf_d = nc.dram_tensor("bf", [1], f32, kind="ExternalInput")
    out_d = nc.dram_tensor("out", [BC, D], f32, kind="ExternalOutput")

    kv = k_d.ap().rearrange("b t d -> (b t) d").rearrange("(n p) d -> p n d", p=128)
    vv = v_d.ap().rearrange("b t d -> (b t) d")

    with tile.TileContext(nc) as tc:
        with (
            tc.tile_pool(name="const", bufs=1) as cpool,
            tc.tile_pool(name="kb", bufs=2) as kbpool,
            tc.tile_pool(name="kt", bufs=2) as ktpool,
            tc.tile_pool(name="vb", bufs=2) as vbpool,
            tc.tile_pool(name="h", bufs=4) as hpool,
            tc.tile_pool(name="sm", bufs=2) as smpool,
            tc.tile_pool(name="p1", bufs=2, space="PSUM") as p1pool,
            tc.tile_pool(name="p2", bufs=2, space="PSUM") as p2pool,
            tc.tile_pool(name="pL", bufs=2, space="PSUM") as pLpool,
            tc.tile_pool(name="pv", bufs=1, space="PSUM") as pvpool,
            tc.tile_pool(name="ps", bufs=1, space="PSUM") as pspool,
        ):
            # ======== one-time setup ========
            ident32 = cpool.tile([128, 128], f32)
            masks.make_identity(nc, ident32[:])
            ident16 = cpool.tile([128, 128], f16)
            nc.vector.tensor_copy(ident16[:], ident32[:])

            # --- W1 pieces ---
            w1a = cpool.tile([64, H1], f32)
            w1b = cpool.tile([64, H1], f32)
            w1c = cpool.tile([64, H1], f32)
            nc.sync.dma_start(w1a[:], W1_d.ap()[0:64, :])
            nc.sync.dma_start(w1b[:], W1_d.ap()[64:128, :])
            nc.sync.dma_start(w1c[:], W1_d.ap()[128:192, :])
            w1s16 = cpool.tile([128, H1], f16)  # [W1b-W1c ; W1d]
            nc.vector.tensor_tensor(w1s16[0:64, :], w1b[:], w1c[:], ALU.subtract)
            nc.gpsimd.dma_start(w1s16[64:128, :], W1_d.ap()[192:256, :])
            w1ac = cpool.tile([64, H1], f32)
            nc.vector.tensor_tensor(w1ac[:], w1a[:], w1c[:], ALU.add)

            w2s16 = cpool.tile([H1, H2], f16)
            nc.gpsimd.dma_start(w2s16[:], W2_d.ap())
            wf16 = cpool.tile([104, 1], f16)
            nc.gpsimd.dma_start(wf16[0:H2, :], Wf_d.ap())
            nc.gpsimd.dma_start(wf16[64:64 + H2, :], Wf_d.ap())
            b2col = cpool.tile([128, 1], f32)
            nc.vector.memset(b2col[:], 0.0)
            nc.sync.dma_start(b2col[0:H2, 0], b2_d.ap())
            nc.sync.dma_start(b2col[64:64 + H2, 0], b2_d.ap())
            b1col = cpool.tile([H1, 1], f32)
            nc.sync.dma_start(b1col[:, 0], b1_d.ap())
            zcol = cpool.tile([128, 1], f32)
            nc.vector.memset(zcol[:], 0.0)
            n8col = cpool.tile([LG_SL, 1], f32)
            nc.vector.memset(n8col[:], -8.0)

            # --- qT (f32), duplicated onto partitions 64-127 ---
            qTall = cpool.tile([128, BC], f32)
            for g in range(2):
                qn = cpool.tile([128, D], f32, tag=f"qn{g}", name=f"qn{g}")
                nc.sync.dma_start(qn[:], q_d.ap()[g * 128:(g + 1) * 128, :])
                pt = pspool.tile([128, 512], f32, tag="setup", name=f"ptq{g}")
                nc.tensor.transpose(pt[0:D, 0:128], qn[:], ident32[:])
                nc.vector.tensor_copy(qTall[0:D, g * 128:(g + 1) * 128],
                                      pt[0:D, 0:128])
            nc.sync.dma_start(qTall[64:64 + D, :], qTall[0:D, :])

            # --- C1T [H1, BC] = (q @ W1ac).T + b1; delta cols for PageIdx ---
            C1T = cpool.tile([H1, BC], f32)
            for g in range(2):
                pt = pspool.tile([128, 512], f32, tag="setup", name=f"ptc{g}")
                nc.tensor.matmul(pt[0:H1, 0:128], w1ac[:],
                                 qTall[0:D, g * 128:(g + 1) * 128],
                                 start=True, stop=True)
                nc.vector.tensor_scalar(C1T[:, g * 128:(g + 1) * 128],
                                        pt[0:H1, 0:128], b1col[:], None, ALU.add)
            dC1T = cpool.tile([H1, BC // 2], f32)
            c1v = C1T[:].rearrange("p (c two) -> p c two", two=2)
            nc.vector.tensor_tensor(dC1T[:], c1v[:, :, 1], c1v[:, :, 0],
                                    ALU.subtract)

            # --- alpha tiles: a1T rep x2 [H1, 400]; a2T pair [104, 400] ---
            a1rep = cpool.tile([H1, CH_ROWS], f16)
            a2rep = cpool.tile([104, CH_ROWS], f16)
            for (ad, hh, dst) in ((a1_d, H1, a1rep), (a2_d, H2, a2rep)):
                an = cpool.tile([128, hh], f32, tag=f"an{hh}", name=f"an{hh}")
                an2 = cpool.tile([72, hh], f32, tag=f"an2{hh}", name=f"an2{hh}")
                nc.sync.dma_start(an[:], ad.ap()[0:128, :])
                nc.sync.dma_start(an2[:], ad.ap()[128:200, :])
                pt = pspool.tile([128, 512], f32, tag="setup", name=f"pta{hh}")
                nc.tensor.transpose(pt[0:hh, 0:128], an[:], ident32[:])
                nc.tensor.transpose(pt[0:hh, 128:200], an2[:], ident32[0:72, 0:72])
                nc.vector.tensor_copy(dst[0:hh, 0:T], pt[0:hh, 0:T])
                nc.vector.tensor_copy(dst[0:hh, T:2 * T], pt[0:hh, 0:T])
                if dst is a2rep:
                    nc.sync.dma_start(dst[64:64 + hh, :], dst[0:hh, :])

            pv = pvpool.tile([128, BC], f32)  # persistent out.T accumulator

            # ======== main loop over super-chunks ========
            for s in range(N_SC):
                # ---- k: cast-load, duplicate cols, transpose blocks ----
                knat = kbpool.tile([128, N_BLK * 128], f16, tag="knat")
                knv = knat[:].rearrange("p (n c) -> p n c", c=128)
                nc.gpsimd.dma_start(knv[:, :, 0:64],
                                    kv[:, s * N_BLK:(s + 1) * N_BLK, :])
                ktw = ktpool.tile([128, SC_ROWS], f16, tag="ktw")
                nc.sync.dma_start(
                    ktw[:].rearrange("p (n c) -> p n c", c=128),
                    knv[:], transpose=True)
                nc.sync.dma_start(ktw[64:128, :], ktw[0:64, :])
                # qk in place on partitions 64-127
                for b in range(SC_B):
                    bg = s * SC_B + b
                    nc.vector.tensor_scalar(
                        ktw[64:128, b * T:(b + 1) * T],
                        ktw[64:128, b * T:(b + 1) * T],
                        qTall[64:128, bg:bg + 1], None, ALU.mult)

                # ---- v: cast-load into padded blocks (128 cols / batch-half) --
                vblk = vbpool.tile([128, SC_B * 256], f16, tag="vblk")
                vbv = vblk[:].rearrange("p (b c) -> p b c", c=256)
                vsrc = v_d.ap()[s * SC_B:(s + 1) * SC_B, :, :]
                nc.gpsimd.dma_start(
                    vbv[:, :, 0:64],
                    vsrc[:, 0:128, :].rearrange("b p d -> p b d"))
                nc.gpsimd.dma_start(
                    vbv[0:72, :, 128:192],
                    vsrc[:, 128:200, :].rearrange("b p d -> p b d"))

                hhsc = []
                for c in range(0, N_CH, 2):
                    h1pair = []
                    for cc in (c, c + 1):
                        p1 = p1pool.tile([H1, CH_ROWS], f32, tag="p1",
                                         name=f"p1_{s}_{cc}")
                        nc.tensor.matmul(p1[:], w1s16[:],
                                         ktw[:, cc * CH_ROWS:(cc + 1) * CH_ROWS],
                                         start=True, stop=True)
                        h1 = hpool.tile([H1, CH_ROWS], f16, tag="h1",
                                        name=f"h1_{s}_{cc}")
                        bg = s * SC_B + cc * CH_B
                        nc.vector._custom_dve(
                            PRELU,
                            out=h1[:].rearrange("p (s n) -> p s n", s=2),
                            in0=p1[:].rearrange("p (s n) -> p s n", s=2),
                            in1=a1rep[:],
                            s0=C1T[:, bg:bg + 1],
                            s1=dC1T[:, bg // 2: bg // 2 + 1])
                        h1pair.append(h1)
                    p2 = p2pool.tile([104, CH_ROWS], f32, tag="p2",
                                     name=f"p2_{s}_{c}")
                    nc.tensor.matmul(p2[0:H2, :], w2s16[:], h1pair[0][:],
                                     start=True, stop=True)
                    nc.tensor.matmul(p2[64:64 + H2, :], w2s16[:], h1pair[1][:],
                                     start=True, stop=True, tile_position=(0, 64))
                    h2 = hpool.tile([104, CH_ROWS], f16, tag="h2",
                                    name=f"h2_{s}_{c}")
                    nc.vector._custom_dve(
                        PRELU,
                        out=h2[:].rearrange("p (s n) -> p s n", s=2),
                        in0=p2[:].rearrange("p (s n) -> p s n", s=2),
                        in1=a2rep[:],
                        s0=b2col[0:104], s1=zcol[0:104])
                    hhsc.append(h2)

                # ---- logits: h2-stationary matmuls, 100-row slices ----
                pL = pLpool.tile([LG_SL, N_LG], f32, tag="pL", name=f"pL{s}")
                for c2 in range(N_CH // 2):
                    h2 = hhsc[c2]
                    for half, base in ((0, 0), (1, 64)):
                        for sl in range(4):
                            gsl = c2 * 8 + half * 4 + sl
                            nc.tensor.matmul(
                                pL[:, gsl:gsl + 1],
                                h2[base:base + H2, sl * LG_SL:(sl + 1) * LG_SL],
                                wf16[base:base + H2, :], start=True, stop=True)

                # ---- softmax in [b, t] layout ----
                u16 = smpool.tile([LG_SL, N_LG], f16, tag="u16", name=f"u{s}")
                nc.scalar.activation(u16[:], pL[:], AF.Exp, bias=n8col[:])
                psm = pspool.tile([128, 1024], f16, tag="setup", name=f"psm{s}")
                nc.tensor.transpose(psm[0:N_LG, 0:LG_SL], u16[:], ident16[0:LG_SL, 0:LG_SL])
                uT = smpool.tile([N_LG, LG_SL], f16, tag="uT", name=f"uT{s}")
                nc.vector.tensor_copy(uT[:], psm[0:N_LG, 0:LG_SL])
                unat = smpool.tile([SC_B, T], f16, tag="unat", name=f"un{s}")
                unat_v = unat[:].rearrange("b (two t) -> b two t", two=2)
                uT_v = uT[:].rearrange("(b two) t -> b two t", b=SC_B)
                nc.sync.dma_start(unat_v[:, 0, :], uT_v[:, 0, :])
                nc.sync.dma_start(unat_v[:, 1, :], uT_v[:, 1, :])
                ssum = smpool.tile([SC_B, 1], f32, tag="ssum", name=f"ss{s}")
                nc.vector.tensor_reduce(ssum[:], unat[:], AX.X, ALU.add)
                rs = smpool.tile([SC_B, 1], f32, tag="rs", name=f"rs{s}")
                nc.vector.reciprocal(rs[:], ssum[:])
                wts = smpool.tile([SC_B, T], f16, tag="wts", name=f"w{s}")
                nc.vector.tensor_scalar(wts[:], unat[:], rs[:], None, ALU.mult)
                psw = pspool.tile([128, 1024], f16, tag="setup", name=f"psw{s}")
                nc.tensor.transpose(psw[0:128, 0:SC_B], wts[:, 0:128], ident16[0:SC_B, 0:SC_B])
                nc.tensor.transpose(psw[0:72, 128:128 + SC_B], wts[:, 128:200],
                                    ident16[0:SC_B, 0:SC_B])
                wT = smpool.tile([128, 2 * SC_B], f16, tag="wT", name=f"wT{s}")
                nc.vector.tensor_copy(wT[:, 0:SC_B], psw[:, 0:SC_B])
                nc.vector.tensor_copy(wT[0:72, SC_B:2 * SC_B],
                                      psw[0:72, 128:128 + SC_B])

                # ---- out accumulation: v-stationary matmuls ----
                for b in range(SC_B):
                    bg = s * SC_B + b
                    nc.tensor.matmul(pv[:, bg:bg + 1],
                                     vblk[:, b * 256:b * 256 + 128],
                                     wT[:, b:b + 1], start=True, stop=False)
                    nc.tensor.matmul(pv[:, bg:bg + 1],
                                     vblk[0:72, b * 256 + 128:b * 256 + 256],
                                     wT[0:72, SC_B + b:SC_B + b + 1],
                                     start=False, stop=True)

            # ======== final: out.T [64, BC] -> [BC, 64] ========
            oT = cpool.tile([64, BC], f32)
            nc.scalar.copy(oT[:], pv[0:64, :])
            for g in range(2):
                pt = pspool.tile([128, 512], f32, tag="setup", name=f"pto{g}")
                nc.tensor.transpose(pt[0:128, 0:64],
                                    oT[:, g * 128:(g + 1) * 128], ident32[0:64, 0:64])
                onat = cpool.tile([128, 64], f32, tag=f"onat{g}", name=f"onat{g}")
                nc.vector.tensor_copy(onat[:], pt[0:128, 0:64])
                nc.sync.dma_start(out_d.ap()[g * 128:(g + 1) * 128, :], onat[:])

    nc.compile()
    _cache["nc"] = nc
    return nc


def kernel(q, k, v, W1, b1, a1, W2, b2, a2, Wf, bf):
    from concourse.bass_utils import run_bass_kernel_spmd

    nc = _build()
    q = np.ascontiguousarray(np.asarray(q, dtype=np.float32))
    k = np.ascontiguousarray(np.asarray(k, dtype=np.float32))
    v = np.ascontiguousarray(np.asarray(v, dtype=np.float32))
    shared = {n: np.ascontiguousarray(np.asarray(x, np.float32))
              for n, x in (("W1", W1), ("b1", b1), ("a1", a1), ("W2", W2),
                           ("b2", b2), ("a2", a2), ("Wf", Wf), ("bf", bf))}
    in_maps = []
    for c in range(N_CORES):
        sl = slice(c * BC, (c + 1) * BC)
        m = {"q": q[sl], "k": k[sl], "v": v[sl]}
        m.update(shared)
        in_maps.append(m)
    res = run_bass_kernel_spmd(nc, in_maps, core_ids=list(range(N_CORES)))
    out = np.empty((B, D), dtype=np.float32)
    for c in range(N_CORES):
        out[c * BC:(c + 1) * BC] = res.results[c]["out"]
    return out
